# revision 8
# baseline (speedup 1.0000x reference)
"""Causal multi-head attention wrapper kernel for Trainium2 (8 NeuronCores).

Sharding: batch (4) x head-group (2 groups of 4 heads) -> 8 cores.
Per core: full attention for one batch element and 4 heads, with
Q/K/V projections column-sharded by head and out-projection row-sharded;
the final out-proj all-reduce (sum over the 2 head-group cores) and the
bias adds that commute with it are done host-side during the gather.

Layout strategy on device (everything fp32r = TF32-rate matmuls):
  - host ships x^T, y^T, wq^T, wk^T, wv^T, wo^T so every matmul operand
    has its contraction dim on partitions (no on-device transposes).
  - K^T [hd, Tk] and V [Tk, hd] per head are built once (stage A).
  - attention computes S^T = K @ Q^T per (head, 256-wide q-chunk) in
    [k-partition, q-free] tiles, exp via ACT (scale folds 1/sqrt(hd)),
    causal masking via additive -1e9 on the two diagonal k-tiles,
    row sums via PE (ones-vector matmul), 1/sum via ACT ln+exp,
    C^T = V^T @ P^T accumulated unnormalized, normalized at the end.
  - probs are written to HBM transposed ([h, k, q]); the host transposes
    back during unshard. Upper-triangle tiles are never written and rely
    on PJRT's zero-initialized output donation.
"""

import os
import sys

import numpy as np

for _p in ("/root/.axon_site/_ro/trn_rl_repo", "/opt/trn_rl_repo"):
    if _p not in sys.path and os.path.isdir(_p):
        sys.path.append(_p)

import concourse.bass as bass
import concourse.mybir as mybir
import concourse.tile as tile

F32 = mybir.dt.float32
F32R = mybir.dt.float32r
EXP = mybir.ActivationFunctionType.Exp
LN = mybir.ActivationFunctionType.Ln
COPY = mybir.ActivationFunctionType.Copy
IDENT = mybir.ActivationFunctionType.Identity

D = 1024            # embed dim
HD = 128            # head dim
NHC = 4             # heads per core
NDT = D // 128      # 8 D-tiles
QC = 256            # q-chunk width
MASK_VAL = -1e9
SCALE = 1.0 / np.sqrt(HD)


def build_nc(T=2048):
    """Build the per-core Bass program (same program on all 8 cores)."""
    NQC = T // QC           # q chunks
    NTT = T // 128          # token tiles
    nc = bass.Bass(target_bir_lowering=False, trn_type="TRN2")

    # ---- DRAM I/O (per-core shapes) ----
    x_t = nc.dram_tensor("x_t", [D, T], F32R, kind="ExternalInput")
    y_t = nc.dram_tensor("y_t", [D, T], F32R, kind="ExternalInput")
    wq_t = nc.dram_tensor("wq_t", [D, NHC * HD], F32R, kind="ExternalInput")
    wk_t = nc.dram_tensor("wk_t", [D, NHC * HD], F32R, kind="ExternalInput")
    wv_t = nc.dram_tensor("wv_t", [D, NHC * HD], F32R, kind="ExternalInput")
    wo_t = nc.dram_tensor("wo_t", [NHC * HD, D], F32R, kind="ExternalInput")
    bq_d = nc.dram_tensor("bq_s", [NHC * HD], F32, kind="ExternalInput")
    probs_t = nc.dram_tensor("probs_t", [NHC, T, T], F32, kind="ExternalOutput")
    out_p = nc.dram_tensor("out_p", [T, D], F32, kind="ExternalOutput")

    with tile.TileContext(nc) as tc:
        import contextlib

        with contextlib.ExitStack() as ctx:
            consts = ctx.enter_context(tc.tile_pool(name="consts", bufs=1))
            wpool = ctx.enter_context(tc.tile_pool(name="wpool", bufs=2))
            xtp = ctx.enter_context(tc.tile_pool(name="xtp", bufs=2))
            kvp = ctx.enter_context(tc.tile_pool(name="kvp", bufs=1))
            qtp = ctx.enter_context(tc.tile_pool(name="qtp", bufs=2))
            ep = ctx.enter_context(tc.tile_pool(name="ep", bufs=2))
            cp = ctx.enter_context(tc.tile_pool(name="cp", bufs=2))
            rbp = ctx.enter_context(tc.tile_pool(name="rbp", bufs=2))
            outp = ctx.enter_context(tc.tile_pool(name="outp", bufs=2))
            # PSUM budget is 8 banks: s(2) + qt/bcast(2) + sum(2) + c/out(2)
            ps_s = ctx.enter_context(tc.tile_pool(name="ps_s", bufs=2, space="PSUM"))
            ps_qt = ctx.enter_context(tc.tile_pool(name="ps_qt", bufs=2, space="PSUM"))
            ps_sum = ctx.enter_context(tc.tile_pool(name="ps_sum", bufs=2, space="PSUM"))
            ps_c = ctx.enter_context(tc.tile_pool(name="ps_c", bufs=2, space="PSUM"))
            ps_b = ps_qt
            ps_o = ps_c

            # ---- constants ----
            ones_f = consts.tile([128, 1], F32, tag="ones_f")
            nc.vector.memset(ones_f, 1.0)
            ones_r = consts.tile([128, 1], F32R, tag="ones_r")
            nc.scalar.activation(out=ones_r, in_=ones_f, func=COPY, scale=1.0)
            onesrow_f = consts.tile([1, 128], F32, tag="onesrow_f")
            nc.vector.memset(onesrow_f, 1.0)
            onesrow_r = consts.tile([1, 128], F32R, tag="onesrow_r")
            nc.scalar.activation(out=onesrow_r, in_=onesrow_f, func=COPY, scale=1.0)
            # additive causal masks for the two diagonal k-tiles of a q-chunk:
            # tile local coords (i=partition=k offset, j=free=q offset);
            # masked (fill) where i + m*128 > j  <=>  keep where j - i - m*128 >= 0
            masks = []
            for m in range(QC // 128):
                mk = consts.tile([128, QC], F32, tag=f"mask{m}")
                nc.gpsimd.memset(mk, 0.0)
                nc.gpsimd.affine_select(
                    out=mk, in_=mk, compare_op=mybir.AluOpType.is_ge,
                    fill=MASK_VAL, base=-m * 128,
                    pattern=[[1, QC]], channel_multiplier=-1,
                )
                masks.append(mk)
            bq_sb = consts.tile([128, NHC], F32, tag="bq")
            nc.sync.dma_start(out=bq_sb, in_=bq_d[:].rearrange("(h p) -> p h", p=128))

            # ---- stage A: K^T and V for all heads ----
            wk_sb = wpool.tile([128, NDT, NHC * HD], F32R, tag="w")
            nc.sync.dma_start(out=wk_sb, in_=wk_t[:, :].rearrange("(t p) m -> p t m", p=128))
            wv_sb = wpool.tile([128, NDT, NHC * HD], F32R, tag="w")
            nc.sync.dma_start(out=wv_sb, in_=wv_t[:, :].rearrange("(t p) m -> p t m", p=128))

            kt_sb = kvp.tile([128, NHC, T], F32R, tag="kt")
            v_sb = kvp.tile([128, NTT, NHC, HD], F32R, tag="v")

            for tci in range(NQC):
                ytr = xtp.tile([128, NDT, QC], F32R, tag="xt")
                nc.sync.dma_start(
                    out=ytr,
                    in_=y_t[:, :].rearrange("(t p) n -> p t n", p=128)[:, :, tci * QC:(tci + 1) * QC],
                )
                for h in range(NHC):
                    kt_ps = ps_s.tile([128, QC], F32, tag="s")
                    for dt in range(NDT):
                        nc.tensor.matmul(
                            kt_ps, wk_sb[:, dt, h * HD:(h + 1) * HD], ytr[:, dt, :],
                            start=(dt == 0), stop=(dt == NDT - 1),
                        )
                    nc.scalar.activation(
                        out=kt_sb[:, h, tci * QC:(tci + 1) * QC], in_=kt_ps,
                        func=COPY, scale=1.0,
                    )
                for tt in range(QC // 128):
                    v_ps = ps_o.tile([128, NHC * HD], F32, tag="o")
                    for dt in range(NDT):
                        nc.tensor.matmul(
                            v_ps, ytr[:, dt, tt * 128:(tt + 1) * 128], wv_sb[:, dt, :],
                            start=(dt == 0), stop=(dt == NDT - 1),
                        )
                    nc.scalar.activation(
                        out=v_sb[:, tci * (QC // 128) + tt, :, :].rearrange("p h d -> p (h d)"),
                        in_=v_ps, func=COPY, scale=1.0,
                    )

            # ---- stage B: attention (+ stage C: out-proj per q-chunk) ----
            wq_sb = wpool.tile([128, NDT, NHC * HD], F32R, tag="w")
            nc.sync.dma_start(out=wq_sb, in_=wq_t[:, :].rearrange("(t p) m -> p t m", p=128))
            wo_sb = wpool.tile([128, NHC, D], F32R, tag="w")
            nc.sync.dma_start(out=wo_sb, in_=wo_t[:, :].rearrange("(h p) o -> p h o", p=128))

            for qc in range(NQC):
                xtr = xtp.tile([128, NDT, QC], F32R, tag="xt")
                nc.sync.dma_start(
                    out=xtr,
                    in_=x_t[:, :].rearrange("(t p) n -> p t n", p=128)[:, :, qc * QC:(qc + 1) * QC],
                )
                c_sb = cp.tile([128, NHC, QC], F32R, tag="c")
                for h in range(NHC):
                    # Q^T chunk
                    qt_ps = ps_qt.tile([128, QC], F32, tag="qt")
                    for dt in range(NDT):
                        nc.tensor.matmul(
                            qt_ps, wq_sb[:, dt, h * HD:(h + 1) * HD], xtr[:, dt, :],
                            start=(dt == 0), stop=(dt == NDT - 1),
                        )
                    qt_sb = qtp.tile([128, QC], F32R, tag="qt")
                    nc.scalar.activation(
                        out=qt_sb, in_=qt_ps, func=IDENT,
                        bias=bq_sb[:, h:h + 1], scale=1.0,
                    )

                    nkt = (qc + 1) * (QC // 128)
                    e_sb = ep.tile([128, nkt, QC], F32, tag="e")
                    c_ps = ps_c.tile([128, QC], F32, tag="c")
                    sum_ps = ps_sum.tile([1, QC], F32, tag="sum")
                    # software pipeline: S(kt+1) issued before C(kt)/sum(kt)
                    for kt in range(nkt):
                        s_ps = ps_s.tile([128, QC], F32, tag="s")
                        nc.tensor.matmul(
                            s_ps, kt_sb[:, h, kt * 128:(kt + 1) * 128], qt_sb,
                            start=True, stop=True,
                        )
                        m = kt - (nkt - len(masks))
                        if m >= 0:
                            nc.vector.tensor_add(s_ps, s_ps, masks[m])
                        nc.scalar.activation(
                            out=e_sb[:, kt, :].bitcast(F32R), in_=s_ps,
                            func=EXP, scale=float(SCALE),
                        )
                        er = e_sb[:, kt, :].bitcast(F32R)
                        nc.tensor.matmul(
                            c_ps, v_sb[:, kt, h, :], er,
                            start=(kt == 0), stop=(kt == nkt - 1),
                        )
                        nc.tensor.matmul(
                            sum_ps, ones_r, er,
                            start=(kt == 0), stop=(kt == nkt - 1),
                        )
                    # 1/sum via ln+exp (same ACT table set as Exp)
                    lns = rbp.tile([1, QC], F32, tag="lns")
                    nc.scalar.activation(out=lns, in_=sum_ps, func=LN, scale=1.0)
                    recip_r = rbp.tile([1, QC], F32R, tag="recip")
                    nc.scalar.activation(out=recip_r, in_=lns, func=EXP, scale=-1.0)
                    b_ps = ps_b.tile([128, QC], F32, tag="b")
                    nc.tensor.matmul(b_ps, onesrow_r, recip_r, start=True, stop=True)
                    rb_sb = rbp.tile([128, QC], F32, tag="rb")
                    nc.scalar.activation(out=rb_sb, in_=b_ps, func=COPY, scale=1.0)
                    # normalize C (psum x rb -> sbuf fp32r)
                    nc.vector.tensor_mul(c_sb[:, h, :], c_ps, rb_sb)
                    # normalize probs tiles in place and ship to HBM
                    # (out dtype fp32r keeps the BIR verifier happy; the
                    # C/sum matmuls only read the pre-normalization values)
                    for kt in range(nkt):
                        nc.vector.tensor_mul(
                            e_sb[:, kt, :].bitcast(F32R), e_sb[:, kt, :], rb_sb
                        )
                    nc.scalar.dma_start(
                        out=probs_t[h, :, :].rearrange("(kt p) q -> p kt q", p=128)[:, 0:nkt, qc * QC:(qc + 1) * QC],
                        in_=e_sb[:, :, :],
                    )

                # out-projection for this q-chunk (accumulate the 4 local heads)
                o_st = outp.tile([128, QC // 128, D], F32, tag="ost")
                for tt in range(QC // 128):
                    for oc in range(D // 512):
                        o_ps = ps_o.tile([128, 512], F32, tag="o")
                        for h in range(NHC):
                            nc.tensor.matmul(
                                o_ps, c_sb[:, h, tt * 128:(tt + 1) * 128],
                                wo_sb[:, h, oc * 512:(oc + 1) * 512],
                                start=(h == 0), stop=(h == NHC - 1),
                            )
                        nc.scalar.activation(
                            out=o_st[:, tt, oc * 512:(oc + 1) * 512], in_=o_ps,
                            func=COPY, scale=1.0,
                        )
                nc.scalar.dma_start(
                    out=out_p[qc * QC:(qc + 1) * QC, :].rearrange("(tt p) o -> p tt o", p=128),
                    in_=o_st,
                )

    from waitsplit_inline import split_waits
    split_waits(nc)
    return nc


# ---------------------------------------------------------------------------
# wait-splitting post-pass (walrus supports very few sync waits per inst)
# ---------------------------------------------------------------------------
_WAITSPLIT_SRC = '''
import concourse.mybir as mybir

def split_waits(nc, limit=1):
    n_added = 0
    for fn in nc.m.functions:
        for blk in fn.blocks:
            insts = blk.instructions
            new_list = []
            changed = False
            for inst in insts:
                si = inst.sync_info
                waits = list(si.on_wait) if si and si.on_wait else []
                if len(waits) > limit:
                    excess = waits[:-limit]
                    keep = waits[-limit:]
                    for w in excess:
                        noop = mybir.InstNoOp(
                            name=nc.get_next_instruction_name(),
                            engine=inst.engine,
                            sync_info=mybir.SyncInfo(on_wait=[w], on_update=[]),
                            bass_nofuse=True,
                        )
                        nc.register_instruction(noop)
                        new_list.append(noop)
                        n_added += 1
                    inst.sync_info = mybir.SyncInfo(
                        on_wait=keep,
                        on_update=list(si.on_update) if si.on_update else [],
                    )
                    changed = True
                new_list.append(inst)
            if changed:
                insts[:] = new_list
    return n_added
'''
import types

waitsplit_inline = types.ModuleType("waitsplit_inline")
exec(_WAITSPLIT_SRC, waitsplit_inline.__dict__)
sys.modules["waitsplit_inline"] = waitsplit_inline


# ---------------------------------------------------------------------------
# host-side sharding / gather
# ---------------------------------------------------------------------------

def _prep_in_maps(x, y, wq, bq, wk, bk, wv, bv, wo, bo, T):
    in_maps = []
    xt_cache = {}
    for b in range(4):
        xt_cache[b] = (
            np.ascontiguousarray(x[b].T),
            np.ascontiguousarray(y[b].T),
        )
    w_cache = {}
    for hg in range(2):
        sl = slice(hg * 512, (hg + 1) * 512)
        w_cache[hg] = dict(
            wq_t=np.ascontiguousarray(wq[sl, :].T),
            wk_t=np.ascontiguousarray(wk[sl, :].T),
            wv_t=np.ascontiguousarray(wv[sl, :].T),
            wo_t=np.ascontiguousarray(wo[:, sl].T),
            bq_s=np.ascontiguousarray(bq[sl]),
        )
    for c in range(8):
        b, hg = c // 2, c % 2
        xt, yt = xt_cache[b]
        m = dict(x_t=xt, y_t=yt)
        m.update(w_cache[hg])
        in_maps.append(m)
    return in_maps


def _gather(results, x, wv, bv, wo, bo, T):
    B = 4
    out = np.zeros((B, T, D), np.float32)
    probs = np.empty((B, 8, T, T), np.float32)
    for c in range(8):
        b, hg = c // 2, c % 2
        out[b] += results[c]["out_p"]
        pt = results[c]["probs_t"]
        for h in range(NHC):
            probs[b, hg * NHC + h] = pt[h].T
    # softmax rows sum to 1, so bv contributes bv @ wo.T to every row;
    # bo and the head-group partial sum also commute to the host.
    out += (bv @ wo.T + bo).astype(np.float32)
    return out, probs


_NC_CACHE = {}


def _get_nc(T):
    if T not in _NC_CACHE:
        _NC_CACHE[T] = build_nc(T)
    return _NC_CACHE[T]


def kernel(x, y, wq, bq, wk, bk, wv, bv, wo, bo):
    x = np.asarray(x, np.float32)
    y = np.asarray(y, np.float32)
    wq = np.asarray(wq, np.float32)
    wk = np.asarray(wk, np.float32)
    wv = np.asarray(wv, np.float32)
    wo = np.asarray(wo, np.float32)
    bq = np.asarray(bq, np.float32)
    bk = np.asarray(bk, np.float32)
    bv = np.asarray(bv, np.float32)
    bo = np.asarray(bo, np.float32)
    T = x.shape[1]
    nc = _get_nc(T)
    in_maps = _prep_in_maps(x, y, wq, bq, wk, bk, wv, bv, wo, bo, T)
    from concourse import bass_utils
    res = bass_utils.run_bass_kernel_spmd(nc, in_maps, core_ids=list(range(8)))
    return _gather(res.results, x, wv, bv, wo, bo, T)


if __name__ == "__main__":
    # tiny smoke: T=512
    T = int(os.environ.get("SMOKE_T", "512"))
    rng = np.random.default_rng(0)
    B = 4
    x = rng.standard_normal((B, T, D), np.float32)
    y = rng.standard_normal((B, T, D), np.float32)
    s = 1.0 / np.sqrt(D)
    wq = (rng.standard_normal((D, D)) * s).astype(np.float32)
    wk = (rng.standard_normal((D, D)) * s).astype(np.float32)
    wv = (rng.standard_normal((D, D)) * s).astype(np.float32)
    wo = (rng.standard_normal((D, D)) * s).astype(np.float32)
    z = np.zeros(D, np.float32)
    out, probs = kernel(x, y, wq, z, wk, z, wv, z, wo, z)
    # numpy reference
    def ref(x, y):
        q = (x @ wq.T).reshape(B, T, 8, 128).transpose(0, 2, 1, 3)
        k = (y @ wk.T).reshape(B, T, 8, 128).transpose(0, 2, 1, 3)
        v = (y @ wv.T).reshape(B, T, 8, 128).transpose(0, 2, 1, 3)
        logits = np.einsum("bhqd,bhkd->bhqk", q, k) / np.sqrt(128)
        mask = np.triu(np.full((T, T), -np.inf, np.float32), k=1)
        logits = logits + mask
        m = logits.max(-1, keepdims=True)
        e = np.exp(logits - m)
        p = e / e.sum(-1, keepdims=True)
        ctx = np.einsum("bhqk,bhkd->bhqd", p, v)
        ctx = ctx.transpose(0, 2, 1, 3).reshape(B, T, D)
        return ctx @ wo.T, p
    r_out, r_p = ref(x, y)
    so = np.abs(r_out).max()
    sp = np.abs(r_p).max()
    print("out absmax relerr:", np.abs(out - r_out).max() / so)
    print("probs absmax relerr:", np.abs(probs - r_p).max() / sp)
    print("masked zeros exact:", np.all(probs[:, :, np.triu_indices(T, 1)[0], np.triu_indices(T, 1)[1]] == 0.0))
